# revision 1
# baseline (speedup 1.0000x reference)
"""Distributed Trainium2 kernel for AttentionLayer+Experts.

Model: B=2, S=2048, D=1024, H=16 heads (DA=64), causal attention with
custom 1/(sqrt(64)*12) scale, residual gate, LayerNorm, then 4
sequence-chunk experts (FFN 1024->4096->1024, exact gelu), residual
with per-expert scalar, per-expert LayerNorm.

Sharding over 8 NeuronCores:
  - Attention: head-parallel. Core c computes heads {2c, 2c+1} for BOTH
    batches (perfect balance, no redundant compute).
  - One 8-rank AllToAll converts head-sharding -> sequence-sharding:
    core c ends up with (batch c//4, seq chunk c%4) which is exactly one
    expert's token chunk, so the expert FFN needs no further comm.
  - Compute in bf16 on TensorE with fp32 accumulation; LayerNorm math in
    fp32. Everything stays feature-major (transposed) so LN/bias terms
    are per-partition; final PE transpose produces the token-major
    output.
  - Softmax denominators ride along in the AV matmul via 64 ones
    columns appended to V (replicated rowsum rows for free), so the
    per-token normalization is 3 full-width DVE ops.
"""

import numpy as np
import ml_dtypes

BF16NP = ml_dtypes.bfloat16

B, S, D, H, DA, E = 2, 2048, 1024, 16, 64, 4
DFF = 4 * D
NCORES = 8
T = S // E  # 512 tokens per chunk / core
P = 128
SCALE = 1.0 / (np.sqrt(DA) * 12.0)
EPS = 1e-5
NDT = D // P      # 8 feature tiles
NQB = S // 512    # 4 query blocks per batch
NKT = S // P      # 16 key tiles per batch
NM1 = DFF // P    # 32 dff tiles
NTT = T // P      # 4 token tiles per chunk

_PROGRAM = None


def _build_program():
    from contextlib import ExitStack
    import concourse.bass as bass
    import concourse.mybir as mybir
    import concourse.tile as tile
    from concourse import bacc

    f32 = mybir.dt.float32
    bf = mybir.dt.bfloat16
    AF = mybir.ActivationFunctionType
    ALU = mybir.AluOpType

    nc = bacc.Bacc("TRN2", target_bir_lowering=False, debug=False,
                   num_devices=NCORES)

    def din(name, shape, dt):
        return nc.dram_tensor(name, shape, dt, kind="ExternalInput").ap()

    xT = din("xT", [B, NDT, P, S], bf)          # x transposed, both batches
    wq = din("wq", [NDT, P, P], bf)             # this core's 2 heads, [k,p,j]
    wk = din("wk", [NDT, P, P], bf)
    wv = din("wv", [NDT, P, P], bf)
    bqv = din("bq", [P, 1], f32)
    bkv = din("bk", [P, 1], f32)
    bvg = din("bvg", [P, 1], f32)               # gate * bv (2 heads)
    gate = din("gate", [P, 1], f32)             # residual gate, replicated
    tri = din("tri", [P, P], bf)                # tri[p,f] = f>=p
    iden = din("iden", [P, P], f32)
    onesc_f = din("onesc_f", [P, 1], f32)
    onesc_b = din("onesc_b", [P, 1], bf)
    onesr_f = din("onesr_f", [1, P], f32)
    xcT = din("xcT", [NDT, P, T], f32)          # residual x^T for my chunk
    lng = din("lng", [P, NDT], f32)
    lnb = din("lnb", [P, NDT], f32)
    w1 = din("w1", [NDT, P, DFF], bf)           # my expert W1 [k,p,m]
    b1v = din("b1", [P, NM1], f32)
    w2 = din("w2", [NM1, P, D], bf)             # my expert W2 [k,p,m]
    b2s = din("b2s", [P, NDT], f32)             # e_scalar * b2
    esv = din("es", [P, 1], f32)                # e_scalar replicated
    elng = din("elng", [P, NDT], f32)
    elnb = din("elnb", [P, NDT], f32)
    out_d = nc.dram_tensor("out", [NTT, P, D], f32, kind="ExternalOutput").ap()

    with tile.TileContext(nc) as tc, ExitStack() as ctx:
        cpool = ctx.enter_context(tc.tile_pool(name="const", bufs=1))
        xtp_ctx = ExitStack()
        xtp = xtp_ctx.enter_context(tc.tile_pool(name="xtp", bufs=2 * NDT))

        # ---- attention-phase inputs first (DMA priority) ----
        wq_sb = cpool.tile([P, NDT, P], bf)
        nc.sync.dma_start(wq_sb[:], wq.rearrange("k p j -> p k j"))
        wk_sb = cpool.tile([P, NDT, P], bf)
        nc.sync.dma_start(wk_sb[:], wk.rearrange("k p j -> p k j"))
        wv_sb = cpool.tile([P, NDT, P], bf)
        nc.sync.dma_start(wv_sb[:], wv.rearrange("k p j -> p k j"))
        bq_sb = cpool.tile([P, 1], f32)
        nc.sync.dma_start(bq_sb[:], bqv[:])
        bk_sb = cpool.tile([P, 1], f32)
        nc.sync.dma_start(bk_sb[:], bkv[:])
        bvg_sb = cpool.tile([P, 1], f32)
        nc.sync.dma_start(bvg_sb[:], bvg[:])
        gate_sb = cpool.tile([P, 1], f32)
        nc.sync.dma_start(gate_sb[:], gate[:])
        tri_sb = cpool.tile([P, P], bf)
        nc.sync.dma_start(tri_sb[:], tri[:])
        xt_all = {}
        for b in range(B):
            for dt in range(NDT):
                t = xtp.tile([P, S], bf, tag="xt", bufs=2 * NDT,
                             name=f"xt{b}_{dt}")
                nc.sync.dma_start(t[:], xT[b, dt])
                xt_all[(b, dt)] = t

        # ---- later-phase constants ----
        iden_sb = cpool.tile([P, P], f32)
        nc.sync.dma_start(iden_sb[:], iden[:])
        onescf_sb = cpool.tile([P, 1], f32)
        nc.sync.dma_start(onescf_sb[:], onesc_f[:])
        onescb_sb = cpool.tile([P, 1], bf)
        nc.sync.dma_start(onescb_sb[:], onesc_b[:])
        onesrf_sb = cpool.tile([1, P], f32)
        nc.sync.dma_start(onesrf_sb[:], onesr_f[:])
        lng_sb = cpool.tile([P, NDT], f32)
        nc.sync.dma_start(lng_sb[:], lng[:])
        lnb_sb = cpool.tile([P, NDT], f32)
        nc.sync.dma_start(lnb_sb[:], lnb[:])
        b1_sb = cpool.tile([P, NM1], f32)
        nc.sync.dma_start(b1_sb[:], b1v[:])
        b2s_sb = cpool.tile([P, NDT], f32)
        nc.sync.dma_start(b2s_sb[:], b2s[:])
        es_sb = cpool.tile([P, 1], f32)
        nc.sync.dma_start(es_sb[:], esv[:])
        elng_sb = cpool.tile([P, NDT], f32)
        nc.sync.dma_start(elng_sb[:], elng[:])
        elnb_sb = cpool.tile([P, NDT], f32)
        nc.sync.dma_start(elnb_sb[:], elnb[:])
        eps_sb = cpool.tile([1, 1], f32)
        nc.vector.memset(eps_sb[:], float(EPS))
        xc_sb = []
        for dt in range(NDT):
            t = cpool.tile([P, T], f32, tag="xc", bufs=NDT, name=f"xc{dt}")
            nc.sync.dma_start(t[:], xcT[dt])
            xc_sb.append(t)

        # a2a DRAM bounce buffers
        dpool = ctx.enter_context(
            tc.tile_pool(name="dramp", bufs=1, space="DRAM"))
        # row 2j+h = head h of chunk j -> shard j is rows [2j, 2j+2)
        a_in = dpool.tile([2 * NCORES, 64, 512], bf, name="a_in")
        a_out = dpool.tile([2 * NCORES, 64, 512], bf, name="a_out")

        # ====== phase 1: projections (both batches), then per-head =======
        # ====== attention sweeps with one AllToAll per head ===============
        with tc.tile_pool(name="psA", bufs=1, space=bass.MemorySpace.PSUM) \
                as psA, \
             tc.tile_pool(name="qkp", bufs=4) as qkp, \
             tc.tile_pool(name="vp", bufs=2 * NKT) as vp, \
             tc.tile_pool(name="ep", bufs=6) as epool, \
             tc.tile_pool(name="stgp", bufs=3) as stgp:
            qTs, kTs, vs = {}, {}, {}
            for b in range(B):
                xt_b = [xt_all[(b, dt)] for dt in range(NDT)]

                # q^T, k^T: [128(2h x 64), S]
                qT = qkp.tile([P, S], bf, tag="qT", bufs=2, name=f"qT{b}")
                kT = qkp.tile([P, S], bf, tag="kT", bufs=2, name=f"kT{b}")
                for (w_sb, b_sb, oT) in ((wq_sb, bq_sb, qT),
                                         (wk_sb, bk_sb, kT)):
                    for qb in range(NQB):
                        q0 = 512 * qb
                        ps = psA.tile([P, 512], f32, tag="proj", bufs=2,
                                      name=f"pj{b}{qb}")
                        for k in range(NDT):
                            nc.tensor.matmul(
                                ps[:], w_sb[:, k, :], xt_b[k][:, q0:q0 + 512],
                                start=(k == 0), stop=(k == NDT - 1))
                        nc.vector.tensor_scalar_add(
                            oT[:, q0:q0 + 512], ps[:], b_sb[:])
                qTs[b], kTs[b] = qT, kT

                # v (token-major), 64 ones columns per head: [128, 2*128]
                # lhsT slice [v_h | ones] makes the AV matmul emit
                # [o^T_h ; rowsum x64] in one go.
                v_b = []
                for tt in range(NKT):
                    t0 = P * tt
                    ps = psA.tile([P, P], f32, tag="proj", bufs=2,
                                  name=f"pv{b}{tt}")
                    for k in range(NDT):
                        nc.tensor.matmul(
                            ps[:], xt_b[k][:, t0:t0 + P], wv_sb[:, k, :],
                            start=(k == 0), stop=(k == NDT - 1))
                    vt = vp.tile([P, 2 * P], bf, tag="v", bufs=2 * NKT,
                                 name=f"v{b}_{tt}")
                    nc.vector.memset(vt[:], 1.0)
                    nc.vector.tensor_copy(vt[:, 0:64], ps[:, 0:64])
                    nc.vector.tensor_copy(vt[:, P:P + 64], ps[:, 64:128])
                    v_b.append(vt)
                vs[b] = v_b

            for h in range(2):
                hp = h * 64
                for b in range(B):
                    qT, kT, v_b = qTs[b], kTs[b], vs[b]
                    for qb in range(NQB):
                        q0 = 512 * qb
                        nkt = 4 * (qb + 1)
                        o_ps = psA.tile([P, 512], f32, tag="o", bufs=2,
                                        name=f"o{b}{qb}{h}")
                        for kt in range(nkt):
                            k0 = P * kt
                            off = max(0, k0 - q0)
                            n = 512 - off
                            s_ps = psA.tile([P, 512], f32, tag="sc", bufs=4,
                                            name=f"s{b}{qb}{h}{kt}")
                            nc.tensor.matmul(
                                s_ps[:, 0:n],
                                kT[hp:hp + 64, k0:k0 + P],
                                qT[hp:hp + 64, q0 + off:q0 + 512],
                                start=True, stop=True)
                            e_sb = epool.tile([P, 512], bf, tag="exp",
                                              bufs=6, name=f"e{b}{qb}{h}{kt}")
                            nc.scalar.activation(
                                e_sb[:, 0:n], s_ps[:, 0:n], AF.Exp,
                                bias=0.0, scale=float(SCALE))
                            if k0 >= q0:  # diagonal block: causal mask
                                nc.vector.tensor_mul(
                                    e_sb[:, 0:P], e_sb[:, 0:P], tri_sb[:])
                            nc.tensor.matmul(
                                o_ps[:, off:512],
                                v_b[kt][:, h * P:(h + 1) * P],
                                e_sb[:, 0:n],
                                start=(kt == 0), stop=(kt == nkt - 1))
                        # bounce rowsum to SBUF: the approx reciprocal's
                        # BITWISE_NOT seed needs raw IEEE fp32 bits, which
                        # the PSUM read path does not guarantee
                        rsum = epool.tile([64, 512], f32, tag="rsum",
                                          bufs=2, name=f"rw{b}{qb}{h}")
                        nc.vector.tensor_copy(rsum[:], o_ps[64:128, :])
                        recip = epool.tile([64, 512], f32, tag="recip",
                                           bufs=2, name=f"rc{b}{qb}{h}")
                        nc.vector.reciprocal_approx_fast(recip[:], rsum[:])
                        stg = stgp.tile([64, 512], bf, tag="stg", bufs=3,
                                        name=f"stg{b}{qb}{h}")
                        # stage = (o * gate) * (1/rowsum) + gate*bv
                        nc.vector.scalar_tensor_tensor(
                            stg[:], o_ps[0:64, :],
                            gate_sb[0:64, :], recip[:], ALU.mult, ALU.mult)
                        nc.vector.tensor_scalar_add(
                            stg[:], stg[:], bvg_sb[hp:hp + 64, :])
                        nc.sync.dma_start(
                            a_in[2 * (b * NQB + qb) + h], stg[:])

            nc.gpsimd.collective_compute(
                "AllToAll", mybir.AluOpType.bypass,
                replica_groups=[list(range(NCORES))],
                ins=[a_in[:].opt()], outs=[a_out[:].opt()])

        xtp_ctx.close()

        # =========== phase 3: residual + LN1 (feature-major) ==========
        x1f = []   # fp32, becomes x1 after LN
        x1b = []   # bf16 copy for FFN rhs
        lnp = ctx.enter_context(tc.tile_pool(name="lnp", bufs=1))
        aop = ctx.enter_context(tc.tile_pool(name="aop", bufs=4))
        smp2 = ctx.enter_context(tc.tile_pool(name="smp2", bufs=1))
        for dt in range(NDT):
            ao = aop.tile([P, 512], bf, tag="ao", name=f"ao{dt}")
            nc.sync.dma_start(ao[:], a_out[2 * dt:2 * dt + 2]
                              .rearrange("a p f -> (a p) f"))
            xf = lnp.tile([P, T], f32, tag="x1f", bufs=NDT, name=f"x1f{dt}")
            nc.vector.tensor_add(xf[:], xc_sb[dt][:], ao[:])
            x1f.append(xf)
            x1b.append(lnp.tile([P, T], bf, tag="x1b", bufs=NDT,
                                name=f"x1b{dt}"))

        def ln_stats_mm(x_tiles, psum_pool, nm, dts):
            """Accumulate sum/sumsq over the given dt tiles (call once per
            dt group; first group allocates)."""
            for dt in dts:
                nc.tensor.matmul(ln_stats_mm.mean[nm][:], onescf_sb[:],
                                 x_tiles[dt][:],
                                 start=(dt == 0), stop=(dt == NDT - 1))
            for dt in dts:
                sq = smp2.tile([P, T], bf, tag="sqt", bufs=3,
                               name=f"sqt{nm}{dt}")
                nc.vector.tensor_mul(sq[:], x_tiles[dt][:], x_tiles[dt][:])
                nc.tensor.matmul(ln_stats_mm.sq[nm][:], onescb_sb[:], sq[:],
                                 start=(dt == 0), stop=(dt == NDT - 1))

        ln_stats_mm.mean = {}
        ln_stats_mm.sq = {}

        def ln_finish(psum_pool, nm):
            """Turn accumulated stats into replicated mu/rsig PSUM tiles."""
            mean_ps, sq_ps = ln_stats_mm.mean[nm], ln_stats_mm.sq[nm]
            mu = smp2.tile([1, 512], f32, tag="sm2", bufs=8, name=f"mu{nm}")
            nc.vector.tensor_scalar_mul(mu[:], mean_ps[:], 1.0 / D)
            ex2 = smp2.tile([1, 512], f32, tag="sm2", bufs=8, name=f"e2{nm}")
            nc.vector.tensor_scalar_mul(ex2[:], sq_ps[:], 1.0 / D)
            mu2 = smp2.tile([1, 512], f32, tag="sm2", bufs=8, name=f"m2{nm}")
            nc.vector.tensor_mul(mu2[:], mu[:], mu[:])
            var = smp2.tile([1, 512], f32, tag="sm2", bufs=8, name=f"vr{nm}")
            nc.vector.tensor_sub(var[:], ex2[:], mu2[:])
            sig = smp2.tile([1, 512], f32, tag="sm2", bufs=8, name=f"sg{nm}")
            nc.scalar.activation(sig[:], var[:], AF.Sqrt, bias=eps_sb[:])
            rsig = smp2.tile([1, 512], f32, tag="sm2", bufs=8,
                             name=f"rs{nm}")
            nc.vector.reciprocal_approx_fast(rsig[:], sig[:])
            mu_rep = psum_pool.tile([P, 512], f32, tag="rep2", bufs=2,
                                    name=f"mr{nm}")
            nc.tensor.matmul(mu_rep[:], onesrf_sb[:], mu[:],
                             start=True, stop=True)
            rs_rep = psum_pool.tile([P, 512], f32, tag="rep2", bufs=2,
                                    name=f"rr{nm}")
            nc.tensor.matmul(rs_rep[:], onesrf_sb[:], rsig[:],
                             start=True, stop=True)
            return mu_rep, rs_rep

        def ln_norm(x, mu_rep, rs_rep, g_sb, b_sb, dt, bf_out,
                    gb_on_act=False):
            nc.vector.tensor_sub(x[:], x[:], mu_rep[:])
            nc.vector.tensor_mul(x[:], x[:], rs_rep[:])
            if gb_on_act:
                nc.scalar.activation(x[:], x[:], AF.Identity,
                                     bias=b_sb[:, dt:dt + 1],
                                     scale=g_sb[:, dt:dt + 1])
            else:
                nc.vector.tensor_scalar(
                    x[:], x[:], g_sb[:, dt:dt + 1], b_sb[:, dt:dt + 1],
                    ALU.mult, ALU.add)
            if bf_out is not None:
                nc.vector.tensor_copy(bf_out[:], x[:])

        with tc.tile_pool(name="psB", bufs=1,
                          space=bass.MemorySpace.PSUM) as psB:
            ln_stats_mm.mean["a"] = psB.tile([1, 512], f32, tag="red",
                                             bufs=2, name="mna")
            ln_stats_mm.sq["a"] = psB.tile([1, 512], f32, tag="red",
                                           bufs=2, name="sqa")
            ln_stats_mm(x1f, psB, "a", range(NDT))
            mu_rep, rs_rep = ln_finish(psB, "a")
            for dt in range(NDT):
                ln_norm(x1f[dt], mu_rep, rs_rep, lng_sb, lnb_sb, dt,
                        x1b[dt], gb_on_act=True)

        # =========== phase 4: expert FFN ==========
        hp_pool = ctx.enter_context(tc.tile_pool(name="hT", bufs=NM1))
        hT = []
        zp = ctx.enter_context(tc.tile_pool(name="zp", bufs=NDT))
        wp = ctx.enter_context(tc.tile_pool(name="wp", bufs=1))
        # FFN1: groups of 4 dff-tiles; stream W1 slices
        with tc.tile_pool(name="psC", bufs=1,
                          space=bass.MemorySpace.PSUM) as psC:
            for mg in range(NM1 // 4):
                w1t = wp.tile([P, NDT, 512], bf, tag="w1", bufs=2,
                              name=f"w1t{mg}")
                nc.sync.dma_start(
                    w1t[:],
                    w1[:, :, mg * 512:(mg + 1) * 512]
                    .rearrange("k p j -> p k j"))
                fps = [psC.tile([P, T], f32, tag="f1", bufs=6,
                                name=f"f1_{mg}_{i}") for i in range(4)]
                for k in range(NDT):
                    for i in range(4):
                        nc.tensor.matmul(
                            fps[i][:], w1t[:, k, i * P:(i + 1) * P],
                            x1b[k][:],
                            start=(k == 0), stop=(k == NDT - 1))
                for i in range(4):
                    m = mg * 4 + i
                    ht = hp_pool.tile([P, T], bf, tag="hT", name=f"hT{m}")
                    nc.scalar.activation(ht[:], fps[i][:], AF.Gelu,
                                         bias=b1_sb[:, m:m + 1], scale=1.0)
                    hT.append(ht)

        # FFN2 in two 4-tile halves so LN2 stats overlap the second half
        z = [None] * NDT
        with tc.tile_pool(name="psE", bufs=1,
                          space=bass.MemorySpace.PSUM) as psE:
            ln_stats_mm.mean["b"] = psE.tile([1, 512], f32, tag="red",
                                             bufs=2, name="mnb")
            ln_stats_mm.sq["b"] = psE.tile([1, 512], f32, tag="red",
                                           bufs=2, name="sqb")
            with tc.tile_pool(name="psD", bufs=1,
                              space=bass.MemorySpace.PSUM) as psD:
                for half in range(2):
                    dts = [half * 4 + i for i in range(4)]
                    yps = [psD.tile([P, T], f32, tag="f2", bufs=4,
                                    name=f"y{dt}") for dt in dts]
                    for k in range(NM1):
                        w2t = wp.tile([P, D // 2], bf, tag="w2", bufs=3,
                                      name=f"w2t{half}_{k}")
                        nc.sync.dma_start(
                            w2t[:], w2[k][:, half * 512:(half + 1) * 512])
                        for i, dt in enumerate(dts):
                            nc.tensor.matmul(
                                yps[i][:], w2t[:, i * P:(i + 1) * P],
                                hT[k][:],
                                start=(k == 0), stop=(k == NM1 - 1))
                    for i, dt in enumerate(dts):
                        zt = zp.tile([P, T], f32, tag="z", bufs=NDT,
                                     name=f"z{dt}")
                        # z = es*y + x1 (+ es*b2)
                        nc.vector.scalar_tensor_tensor(
                            zt[:], yps[i][:], es_sb[:], x1f[dt][:],
                            ALU.mult, ALU.add)
                        nc.vector.tensor_scalar_add(
                            zt[:], zt[:], b2s_sb[:, dt:dt + 1])
                        z[dt] = zt
                    # LN2 stats for this half overlap the next half's MMs
                    ln_stats_mm(z, psE, "b", dts)

            # =========== phase 5: LN2 + transpose + output ==========
            mu2r, rs2r = ln_finish(psE, "b")
            with tc.tile_pool(name="psF", bufs=1,
                              space=bass.MemorySpace.PSUM) as psF, \
                 tc.tile_pool(name="outp", bufs=NTT) as outp:
                ot = [outp.tile([P, D], f32, tag="ot", name=f"ot{tt}")
                      for tt in range(NTT)]
                for dt in range(NDT):
                    ln_norm(z[dt], mu2r, rs2r, elng_sb, elnb_sb, dt, None,
                            gb_on_act=True)
                    for tt in range(NTT):
                        tp = psF.tile([P, P], f32, tag="tr", bufs=4,
                                      name=f"tr{tt}{dt}")
                        nc.tensor.transpose(
                            tp[:], z[dt][:, tt * P:(tt + 1) * P], iden_sb[:])
                        dst = ot[tt][:, dt * P:(dt + 1) * P]
                        if (dt + tt) % 2:
                            nc.scalar.copy(dst, tp[:])
                        else:
                            nc.vector.tensor_copy(dst, tp[:])
                for tt in range(NTT):
                    nc.sync.dma_start(out_d[tt], ot[tt][:])

    nc.compile()
    return nc


def _get_program():
    global _PROGRAM
    if _PROGRAM is None:
        _PROGRAM = _build_program()
    return _PROGRAM


def _host_prep(inputs):
    """Shard + lay out inputs for each of the 8 cores."""
    x = np.asarray(inputs["x"], np.float32)
    Wq = np.asarray(inputs["Wq"], np.float32)
    bq = np.asarray(inputs["bq"], np.float32)
    Wk = np.asarray(inputs["Wk"], np.float32)
    bk = np.asarray(inputs["bk"], np.float32)
    Wv = np.asarray(inputs["Wv"], np.float32)
    bv = np.asarray(inputs["bv"], np.float32)
    scalar = np.float32(inputs["scalar"])
    ln_g = np.asarray(inputs["ln_g"], np.float32)
    ln_b = np.asarray(inputs["ln_b"], np.float32)
    eW1 = np.asarray(inputs["eW1"], np.float32)
    eb1 = np.asarray(inputs["eb1"], np.float32)
    eW2 = np.asarray(inputs["eW2"], np.float32)
    eb2 = np.asarray(inputs["eb2"], np.float32)
    e_scalar = np.asarray(inputs["e_scalar"], np.float32)
    eln_g = np.asarray(inputs["eln_g"], np.float32)
    eln_b = np.asarray(inputs["eln_b"], np.float32)

    xT_all = np.ascontiguousarray(x.transpose(0, 2, 1)).reshape(B, NDT, P, S)
    xT_bf = xT_all.astype(BF16NP)
    tri = (np.arange(P)[None, :] >= np.arange(P)[:, None])
    iden = np.eye(P, dtype=np.float32)

    def col(v):
        return np.ascontiguousarray(v.reshape(-1, 1), dtype=np.float32)

    def pk(v):  # [D]-like -> [P, n]
        n = v.size // P
        return np.ascontiguousarray(v.reshape(n, P).T, dtype=np.float32)

    in_maps = []
    for c in range(NCORES):
        h0 = 2 * c
        b_out, e_out = c // NQB, c % NQB
        t0 = e_out * T
        wq_c = np.concatenate([Wq[h0], Wq[h0 + 1]], axis=1)  # [1024,128]
        wk_c = np.concatenate([Wk[h0], Wk[h0 + 1]], axis=1)
        wv_c = np.concatenate([Wv[h0], Wv[h0 + 1]], axis=1)
        bq_c = np.concatenate([bq[h0], bq[h0 + 1]])
        bk_c = np.concatenate([bk[h0], bk[h0 + 1]])
        bv_c = np.concatenate([bv[h0], bv[h0 + 1]])
        xc = np.ascontiguousarray(x[b_out, t0:t0 + T, :].T)  # [1024, 512]
        m = {
            "xT": xT_bf,
            "wq": np.ascontiguousarray(wq_c.reshape(NDT, P, P), BF16NP),
            "wk": np.ascontiguousarray(wk_c.reshape(NDT, P, P), BF16NP),
            "wv": np.ascontiguousarray(wv_c.reshape(NDT, P, P), BF16NP),
            "bq": col(bq_c),
            "bk": col(bk_c),
            "bvg": col(scalar * bv_c),
            "gate": np.full((P, 1), scalar, np.float32),
            "tri": tri.astype(BF16NP),
            "iden": iden,
            "onesc_f": np.ones((P, 1), np.float32),
            "onesc_b": np.ones((P, 1), BF16NP),
            "onesr_f": np.ones((1, P), np.float32),
            "xcT": np.ascontiguousarray(xc.reshape(NDT, P, T), np.float32),
            "lng": pk(ln_g),
            "lnb": pk(ln_b),
            "w1": np.ascontiguousarray(
                eW1[e_out].reshape(NDT, P, DFF), BF16NP),
            "b1": pk(eb1[e_out]),
            "w2": np.ascontiguousarray(
                eW2[e_out].reshape(NM1, P, D), BF16NP),
            "b2s": pk(e_scalar[e_out] * eb2[e_out]),
            "es": np.full((P, 1), e_scalar[e_out], np.float32),
            "elng": pk(eln_g[e_out]),
            "elnb": pk(eln_b[e_out]),
        }
        in_maps.append(m)
    return in_maps


_LAST_RESULT = {}


def kernel(**inputs) -> np.ndarray:
    import os
    from concourse.bass_utils import run_bass_kernel_spmd

    nc = _get_program()
    in_maps = _host_prep(inputs)
    trace = bool(int(os.environ.get("KBENCH_TRACE", "0")))
    res = run_bass_kernel_spmd(nc, in_maps, core_ids=list(range(NCORES)),
                               trace=trace)
    _LAST_RESULT["exec_time_ns"] = res.exec_time_ns
    _LAST_RESULT["res"] = res

    out = np.empty((B, S, D), np.float32)
    for c in range(NCORES):
        b_out, e_out = c // NQB, c % NQB
        chunk = np.asarray(res.results[c]["out"], np.float32)
        out[b_out, e_out * T:(e_out + 1) * T, :] = chunk.reshape(T, D)
    return out



# revision 9
# speedup vs baseline: 1.5294x; 1.5294x over previous
"""Distributed Trainium2 kernel for AttentionLayer+Experts (fp8 rebuild).

Model: B=2, S=2048, D=1024, H=16 heads (DA=64), causal attention with
custom 1/(sqrt(64)*12) scale, residual gate, LayerNorm, then 4
sequence-chunk experts (FFN 1024->4096->1024, exact gelu), residual
with per-expert scalar, per-expert LayerNorm.

Sharding over 8 NeuronCores (unchanged from baseline):
  - Attention head-parallel (core c owns heads 2c, 2c+1 for both
    batches); AllToAll converts head-sharding -> sequence-sharding so
    core c ends up with (batch c//4, seq chunk c%4) = one expert chunk.

Perf design vs baseline:
  - All wide-contraction matmuls (QKV projections, AV, FFN1, FFN2) run
    in fp8e4m3 with MatmulPerfMode.DoubleRow: two 128-row k-subtiles
    per instruction at 2x column rate. Operands are laid out in
    [128, 2, n] "pair" tiles. Scores stay bf16 (64-wide contraction
    cannot pair).
  - The AllToAll is split in two (by head parity) so the first overlaps
    the second head's attention sweep, and its payload is fp8 with a
    16x prescale (values sit near e4m3's subnormal edge otherwise).
  - LN1's gamma/beta are folded into W1/b1 and the z-residual, so the
    normalized activations go straight to fp8 pair tiles.
  - LN stats matmuls run in bf16 (fp32 matmul is 4 cyc/col).
  - Output leaves the device feature-major bf16; the host transposes.
"""

import numpy as np
import ml_dtypes

BF16NP = ml_dtypes.bfloat16
F8NP = ml_dtypes.float8_e4m3

B, S, D, H, DA, E = 2, 2048, 1024, 16, 64, 4
DFF = 4 * D
NCORES = 8
T = S // E        # 512 tokens per chunk / core
P = 128
SCALE = 1.0 / (np.sqrt(DA) * 12.0)
EPS = 1e-5
NDT = D // P      # 8 feature tiles
NPD = NDT // 2    # 4 feature pair-tiles
NQB = S // 512    # 4 query blocks per batch
NKT = S // P      # 16 key tiles per batch
NM1 = DFF // P    # 32 dff tiles
NM1P = NM1 // 2   # 16 dff pair-tiles
PRE = 16.0        # a2a fp8 prescale

_PROGRAM = None


def _build_program():
    from contextlib import ExitStack
    import concourse.bass as bass
    import concourse.mybir as mybir
    import concourse.tile as tile
    from concourse import bacc

    f32 = mybir.dt.float32
    bf = mybir.dt.bfloat16
    f8 = mybir.dt.float8e4
    AF = mybir.ActivationFunctionType
    ALU = mybir.AluOpType
    DR = mybir.MatmulPerfMode.DoubleRow

    nc = bacc.Bacc("TRN2", target_bir_lowering=False, debug=False,
                   num_devices=NCORES)

    def din(name, shape, dt):
        return nc.dram_tensor(name, shape, dt, kind="ExternalInput").ap()

    xp_d = din("xp", [B, NPD, P, 2, S], f8)      # x pair-tiled, both batches
    wq = din("wq", [NPD, P, 2, P], f8)           # this core's 2 heads
    wk = din("wk", [NPD, P, 2, P], f8)
    wv = din("wv", [NPD, P, 2, P], f8)
    bqv = din("bq", [P, 1], f32)
    bkv = din("bk", [P, 1], f32)
    bvg16 = din("bvg16", [P, 1], f32)            # PRE * gate * bv
    gate16 = din("gate16", [P, 1], f32)          # PRE * gate
    tri = din("tri", [P, P], f8)                 # tri[p,f] = f>=p
    onesc_b = din("onesc_b", [P, 1], bf)
    onesr_f = din("onesr_f", [1, P], f32)
    xcT = din("xcT", [NDT, P, T], f32)           # residual x^T for my chunk
    lng = din("lng", [P, NDT], f32)              # ln1 gamma (per dt col)
    zbias = din("zbias", [P, NDT], f32)          # ln1 beta + es*b2
    w1 = din("w1", [NPD, P, 2, DFF], f8)         # g-folded W1 pair tiles
    b1v = din("b1", [P, NM1], f32)               # b1 + ln1beta @ W1
    w2 = din("w2", [NM1P, P, 2, D], f8)
    esv = din("es", [P, 1], f32)                 # e_scalar replicated
    elng = din("elng", [P, NDT], f32)
    elnb = din("elnb", [P, NDT], f32)
    out_d = nc.dram_tensor("out", [NDT, P, T], bf, kind="ExternalOutput").ap()

    with tile.TileContext(nc) as tc, ExitStack() as ctx:
        cpool = ctx.enter_context(tc.tile_pool(name="const", bufs=1))
        wpool = ctx.enter_context(tc.tile_pool(name="wpool", bufs=1))
        xcp = ctx.enter_context(tc.tile_pool(name="xcp", bufs=NDT))
        dpool = ctx.enter_context(
            tc.tile_pool(name="dramp", bufs=1, space="DRAM"))
        qkp_ctx = ExitStack()
        qkp = qkp_ctx.enter_context(tc.tile_pool(name="qkp", bufs=4))
        vp_ctx = ExitStack()
        vp = vp_ctx.enter_context(tc.tile_pool(name="vp", bufs=NKT))
        xtp_ctx = ExitStack()
        xtp = xtp_ctx.enter_context(tc.tile_pool(name="xtp", bufs=2 * NPD))

        # ---- attention-phase inputs first (DMA priority) ----
        wq_sb = cpool.tile([P, NPD, 2, P], f8)
        nc.sync.dma_start(wq_sb[:], wq.rearrange("k p j m -> p k j m"))
        wk_sb = cpool.tile([P, NPD, 2, P], f8)
        nc.sync.dma_start(wk_sb[:], wk.rearrange("k p j m -> p k j m"))
        wv_sb = cpool.tile([P, NPD, 2, P], f8)
        nc.sync.dma_start(wv_sb[:], wv.rearrange("k p j m -> p k j m"))
        bq_sb = cpool.tile([P, 1], f32)
        nc.sync.dma_start(bq_sb[:], bqv[:])
        bk_sb = cpool.tile([P, 1], f32)
        nc.sync.dma_start(bk_sb[:], bkv[:])
        bvg_sb = cpool.tile([P, 1], f32)
        nc.sync.dma_start(bvg_sb[:], bvg16[:])
        gate_sb = cpool.tile([P, 1], f32)
        nc.sync.dma_start(gate_sb[:], gate16[:])
        tri_sb = cpool.tile([P, P], f8)
        nc.sync.dma_start(tri_sb[:], tri[:])
        xt_all = {}
        for b in range(B):
            for pt in range(NPD):
                t = xtp.tile([P, 2, S], f8, tag="xt", bufs=2 * NPD,
                             name=f"xt{b}_{pt}")
                nc.sync.dma_start(t[:], xp_d[b, pt])
                xt_all[(b, pt)] = t

        # ---- later-phase constants + FFN weight prefetch ----
        onescb_sb = cpool.tile([P, 1], bf)
        nc.sync.dma_start(onescb_sb[:], onesc_b[:])
        onesrf_sb = cpool.tile([1, P], f32)
        nc.sync.dma_start(onesrf_sb[:], onesr_f[:])
        lng_sb = cpool.tile([P, NDT], f32)
        nc.sync.dma_start(lng_sb[:], lng[:])
        zbias_sb = cpool.tile([P, NDT], f32)
        nc.sync.dma_start(zbias_sb[:], zbias[:])
        b1_sb = cpool.tile([P, NM1], f32)
        nc.sync.dma_start(b1_sb[:], b1v[:])
        es_sb = cpool.tile([P, 1], f32)
        nc.sync.dma_start(es_sb[:], esv[:])
        elng_sb = cpool.tile([P, NDT], f32)
        nc.sync.dma_start(elng_sb[:], elng[:])
        elnb_sb = cpool.tile([P, NDT], f32)
        nc.sync.dma_start(elnb_sb[:], elnb[:])
        eps_sb = cpool.tile([1, 1], f32)
        nc.vector.memset(eps_sb[:], float(EPS))
        xc_sb = []
        for dt in range(NDT):
            t = xcp.tile([P, T], f32, tag="xc", bufs=NDT, name=f"xc{dt}")
            nc.sync.dma_start(t[:], xcT[dt])
            xc_sb.append(t)
        w1_sb = []
        for pt in range(NPD):
            t = wpool.tile([P, 2, DFF], f8, tag="w1", bufs=NPD,
                           name=f"w1_{pt}")
            nc.sync.dma_start(t[:], w1[pt])
            w1_sb.append(t)
        w2_sb = []
        for kp in range(NM1P):
            t = wpool.tile([P, 2, D], f8, tag="w2", bufs=NM1P,
                           name=f"w2_{kp}")
            nc.sync.dma_start(t[:], w2[kp])
            w2_sb.append(t)

        # a2a DRAM bounce buffers (split by head parity, fp8 payload)
        a_in = [dpool.tile([NCORES, 64, 512], f8, name=f"a_in{h}")
                for h in range(2)]
        a_out = [dpool.tile([NCORES, 64, 512], f8, name=f"a_out{h}")
                 for h in range(2)]

        # ====== phase 1: projections (fp8 DoubleRow) =====================
        qTs, kTs, vs = {}, {}, {}
        with tc.tile_pool(name="psP", bufs=1,
                          space=bass.MemorySpace.PSUM) as psP:
            for b in range(B):
                qT = qkp.tile([P, S], bf, tag="qT", bufs=2, name=f"qT{b}")
                kT = qkp.tile([P, S], bf, tag="kT", bufs=2, name=f"kT{b}")
                # q,k: pair-outer accumulation; 8 psum banks
                ps_q = [psP.tile([P, 512], f32, tag="pj", bufs=8,
                                 name=f"pq{b}{qb}") for qb in range(NQB)]
                ps_k = [psP.tile([P, 512], f32, tag="pj", bufs=8,
                                 name=f"pk{b}{qb}") for qb in range(NQB)]
                for pt in range(NPD):
                    xt = xt_all[(b, pt)]
                    for qb in range(NQB):
                        q0 = 512 * qb
                        nc.tensor.matmul(
                            ps_q[qb][:], wq_sb[:, pt], xt[:, :, q0:q0 + 512],
                            start=(pt == 0), stop=(pt == NPD - 1),
                            perf_mode=DR)
                        nc.tensor.matmul(
                            ps_k[qb][:], wk_sb[:, pt], xt[:, :, q0:q0 + 512],
                            start=(pt == 0), stop=(pt == NPD - 1),
                            perf_mode=DR)
                for qb in range(NQB):
                    q0 = 512 * qb
                    nc.vector.tensor_scalar_add(
                        qT[:, q0:q0 + 512], ps_q[qb][:], bq_sb[:])
                    nc.vector.tensor_scalar_add(
                        kT[:, q0:q0 + 512], ps_k[qb][:], bk_sb[:])
                qTs[b], kTs[b] = qT, kT

                # v token-major in pair tiles [tok, ktpar, vd|ones]
                v_b = []
                for ktp in range(NKT // 2):
                    vt = vp.tile([P, 2, 2 * P], f8, tag="v", bufs=NKT,
                                 name=f"v{b}_{ktp}")
                    nc.vector.memset(vt[:], 1.0)
                    v_b.append(vt)
                for tt in range(NKT):
                    t0 = P * tt
                    # [P,512]-shaped slot keeps each group bank-aligned
                    pv = psP.tile([P, 512], f32, tag="pj", bufs=8,
                                  name=f"pv{b}{tt}")
                    for pt in range(NPD):
                        nc.tensor.matmul(
                            pv[:, 0:P],
                            xt_all[(b, pt)][:, :, t0:t0 + P], wv_sb[:, pt],
                            start=(pt == 0), stop=(pt == NPD - 1),
                            perf_mode=DR)
                    vt = v_b[tt // 2]
                    nc.vector.tensor_copy(vt[:, tt % 2, 0:64], pv[:, 0:64])
                    nc.vector.tensor_copy(vt[:, tt % 2, P:P + 64],
                                          pv[:, 64:128])
                vs[b] = v_b
        xtp_ctx.close()

        # ====== phase 2: attention sweeps; a2a per head parity ===========
        with tc.tile_pool(name="psA", bufs=1,
                          space=bass.MemorySpace.PSUM) as psA, \
             tc.tile_pool(name="ep", bufs=6) as epool, \
             tc.tile_pool(name="stgp", bufs=3) as stgp:
            for h in range(2):
                hp = h * 64
                for b in range(B):
                    qT, kT, v_b = qTs[b], kTs[b], vs[b]
                    for qb in range(NQB):
                        q0 = 512 * qb
                        npair = 2 * qb + 2
                        o_ps = psA.tile([P, 512], f32, tag="o", bufs=2,
                                        name=f"o{b}{qb}{h}")
                        for ktp in range(npair):
                            e_pair = epool.tile([P, 2, 512], f8, tag="exp",
                                                bufs=4,
                                                name=f"e{b}{qb}{h}{ktp}")
                            for j in range(2):
                                kt = 2 * ktp + j
                                k0 = P * kt
                                off = max(0, k0 - q0)
                                s_ps = psA.tile([P, 512], f32, tag="sc",
                                                bufs=4,
                                                name=f"s{b}{qb}{h}{kt}")
                                nc.tensor.matmul(
                                    s_ps[:, off:512],
                                    kT[hp:hp + 64, k0:k0 + P],
                                    qT[hp:hp + 64, q0 + off:q0 + 512],
                                    start=True, stop=True)
                                if off:
                                    nc.vector.memset(
                                        e_pair[:, j, 0:off], 0.0)
                                nc.scalar.activation(
                                    e_pair[:, j, off:512], s_ps[:, off:512],
                                    AF.Exp, bias=0.0, scale=float(SCALE))
                                if k0 >= q0:  # diagonal block: causal mask
                                    nc.vector.tensor_mul(
                                        e_pair[:, j, off:off + P],
                                        e_pair[:, j, off:off + P], tri_sb[:])
                            nc.tensor.matmul(
                                o_ps[:],
                                v_b[ktp][:, :, h * 2 * 64:h * 2 * 64 + P],
                                e_pair[:],
                                start=(ktp == 0), stop=(ktp == npair - 1),
                                perf_mode=DR)
                        # rowsum -> SBUF (reciprocal seed needs IEEE fp32)
                        rsum = epool.tile([64, 512], f32, tag="rsum",
                                          bufs=2, name=f"rw{b}{qb}{h}")
                        nc.vector.tensor_copy(rsum[:], o_ps[64:128, :])
                        recip = epool.tile([64, 512], f32, tag="recip",
                                           bufs=2, name=f"rc{b}{qb}{h}")
                        nc.vector.reciprocal_approx_fast(recip[:], rsum[:])
                        stgb = stgp.tile([64, 512], bf, tag="stgb", bufs=3,
                                         name=f"sb{b}{qb}{h}")
                        # stage = (o * 16gate) * (1/rowsum) + 16*gate*bv
                        nc.vector.scalar_tensor_tensor(
                            stgb[:], o_ps[0:64, :],
                            gate_sb[0:64, :], recip[:], ALU.mult, ALU.mult)
                        stg = stgp.tile([64, 512], f8, tag="stg", bufs=3,
                                        name=f"stg{b}{qb}{h}")
                        nc.vector.tensor_scalar_add(
                            stg[:], stgb[:], bvg_sb[hp:hp + 64, :])
                        nc.sync.dma_start(a_in[h][b * NQB + qb], stg[:])
                nc.gpsimd.collective_compute(
                    "AllToAll", mybir.AluOpType.bypass,
                    replica_groups=[list(range(NCORES))],
                    ins=[a_in[h][:].opt()], outs=[a_out[h][:].opt()])
        vp_ctx.close()
        qkp_ctx.close()

        # =========== phase 3: residual + LN1 (gamma/beta folded) ==========
        x1f = []   # fp32; becomes u = (x1-mu)*rsig after norm
        x1p = []   # fp8 pair tiles for FFN1 rhs
        lnp = ctx.enter_context(tc.tile_pool(name="lnp", bufs=1))
        aop = ctx.enter_context(tc.tile_pool(name="aop", bufs=4))
        smp2 = ctx.enter_context(tc.tile_pool(name="smp2", bufs=1))
        for pt in range(NPD):
            x1p.append(lnp.tile([P, 2, T], f8, tag="x1p", bufs=NPD,
                                name=f"x1p{pt}"))
        for dt in range(NDT):
            ao = aop.tile([P, 512], f8, tag="ao", name=f"ao{dt}")
            nc.sync.dma_start(ao[0:64, :], a_out[0][dt])
            nc.sync.dma_start(ao[64:128, :], a_out[1][dt])
            xf = lnp.tile([P, T], f32, tag="x1f", bufs=NDT, name=f"x1f{dt}")
            # x1 = xc + stage/16
            nc.vector.scalar_tensor_tensor(
                xf[:], ao[:], 1.0 / PRE, xc_sb[dt][:], ALU.mult, ALU.add)
            x1f.append(xf)

        def ln_stats(x_tile, mean_ps, sq_ps, dt, nm):
            xc16 = smp2.tile([P, T], bf, tag="xc16", bufs=3,
                             name=f"xc16{nm}{dt}")
            nc.vector.tensor_copy(xc16[:], x_tile[:])
            nc.tensor.matmul(mean_ps[:], onescb_sb[:], xc16[:],
                             start=(dt == 0), stop=(dt == NDT - 1))
            sq = smp2.tile([P, T], bf, tag="sqt", bufs=3,
                           name=f"sqt{nm}{dt}")
            nc.vector.tensor_mul(sq[:], xc16[:], xc16[:])
            nc.tensor.matmul(sq_ps[:], onescb_sb[:], sq[:],
                             start=(dt == 0), stop=(dt == NDT - 1))

        def ln_finish(mean_ps, sq_ps, psum_pool, nm):
            """Replicated mu/rsig PSUM tiles from accumulated stats."""
            mu = smp2.tile([1, 512], f32, tag="sm2", bufs=8, name=f"mu{nm}")
            nc.vector.tensor_scalar_mul(mu[:], mean_ps[:], 1.0 / D)
            ex2 = smp2.tile([1, 512], f32, tag="sm2", bufs=8, name=f"e2{nm}")
            nc.vector.tensor_scalar_mul(ex2[:], sq_ps[:], 1.0 / D)
            mu2 = smp2.tile([1, 512], f32, tag="sm2", bufs=8, name=f"m2{nm}")
            nc.vector.tensor_mul(mu2[:], mu[:], mu[:])
            var = smp2.tile([1, 512], f32, tag="sm2", bufs=8, name=f"vr{nm}")
            nc.vector.tensor_sub(var[:], ex2[:], mu2[:])
            sig = smp2.tile([1, 512], f32, tag="sm2", bufs=8, name=f"sg{nm}")
            nc.scalar.activation(sig[:], var[:], AF.Sqrt, bias=eps_sb[:])
            rsig = smp2.tile([1, 512], f32, tag="sm2", bufs=8,
                             name=f"rs{nm}")
            nc.vector.reciprocal_approx_fast(rsig[:], sig[:])
            mu_rep = psum_pool.tile([P, 512], f32, tag="rep2", bufs=2,
                                    name=f"mr{nm}")
            nc.tensor.matmul(mu_rep[:], onesrf_sb[:], mu[:],
                             start=True, stop=True)
            rs_rep = psum_pool.tile([P, 512], f32, tag="rep2", bufs=2,
                                    name=f"rr{nm}")
            nc.tensor.matmul(rs_rep[:], onesrf_sb[:], rsig[:],
                             start=True, stop=True)
            return mu_rep, rs_rep

        with tc.tile_pool(name="psB", bufs=1,
                          space=bass.MemorySpace.PSUM) as psB:
            mean_a = psB.tile([1, 512], f32, tag="red", bufs=2, name="mna")
            sq_a = psB.tile([1, 512], f32, tag="red", bufs=2, name="sqa")
            for dt in range(NDT):
                ln_stats(x1f[dt], mean_a, sq_a, dt, "a")
            mu_rep, rs_rep = ln_finish(mean_a, sq_a, psB, "a")
            for dt in range(NDT):
                nc.vector.tensor_sub(x1f[dt][:], x1f[dt][:], mu_rep[:])
                nc.vector.tensor_mul(x1f[dt][:], x1f[dt][:], rs_rep[:])
                nc.vector.tensor_copy(x1p[dt // 2][:, dt % 2, :],
                                      x1f[dt][:])

            # =========== phase 4: expert FFN1 (fp8 DR) ==========
            hp_pool = ctx.enter_context(tc.tile_pool(name="hT", bufs=NM1P))
            hT = [hp_pool.tile([P, 2, T], f8, tag="hT", name=f"hT{kp}")
                  for kp in range(NM1P)]
            with tc.tile_pool(name="psC", bufs=1,
                              space=bass.MemorySpace.PSUM) as psC:
                for mg in range(11):
                    ms = range(3 * mg, min(3 * mg + 3, NM1))
                    fps = {m: psC.tile([P, T], f32, tag="f1", bufs=3,
                                       name=f"f1_{m}") for m in ms}
                    for pt in range(NPD):
                        for m in ms:
                            nc.tensor.matmul(
                                fps[m][:],
                                w1_sb[pt][:, :, m * P:(m + 1) * P],
                                x1p[pt][:],
                                start=(pt == 0), stop=(pt == NPD - 1),
                                perf_mode=DR)
                    for m in ms:
                        nc.scalar.activation(
                            hT[m // 2][:, m % 2, :], fps[m][:], AF.Gelu,
                            bias=b1_sb[:, m:m + 1], scale=1.0)

        # =========== phase 5: FFN2 (fp8 DR) + LN2 ==========
        z = [None] * NDT
        zp = ctx.enter_context(tc.tile_pool(name="zp", bufs=NDT))
        with tc.tile_pool(name="psE", bufs=1,
                          space=bass.MemorySpace.PSUM) as psE:
            mean_b = psE.tile([1, 512], f32, tag="red", bufs=2, name="mnb")
            sq_b = psE.tile([1, 512], f32, tag="red", bufs=2, name="sqb")
            with tc.tile_pool(name="psD", bufs=1,
                              space=bass.MemorySpace.PSUM) as psD:
                for dg in range(3):
                    dts = range(3 * dg, min(3 * dg + 3, NDT))
                    yps = {dt: psD.tile([P, T], f32, tag="f2", bufs=3,
                                        name=f"y{dt}") for dt in dts}
                    for kp in range(NM1P):
                        for dt in dts:
                            nc.tensor.matmul(
                                yps[dt][:],
                                w2_sb[kp][:, :, dt * P:(dt + 1) * P],
                                hT[kp][:],
                                start=(kp == 0), stop=(kp == NM1P - 1),
                                perf_mode=DR)
                    for dt in dts:
                        # z = es*y + (ln1b + es*b2) + ln1g*u
                        tz = smp2.tile([P, T], f32, tag="tz", bufs=3,
                                       name=f"tz{dt}")
                        nc.scalar.activation(
                            tz[:], yps[dt][:], AF.Identity,
                            bias=zbias_sb[:, dt:dt + 1],
                            scale=es_sb[:])
                        zt = zp.tile([P, T], f32, tag="z", bufs=NDT,
                                     name=f"z{dt}")
                        nc.vector.scalar_tensor_tensor(
                            zt[:], x1f[dt][:], lng_sb[:, dt:dt + 1], tz[:],
                            ALU.mult, ALU.add)
                        z[dt] = zt
                        ln_stats(zt, mean_b, sq_b, dt, "b")

            # =========== phase 6: LN2 + output (feature-major) ==========
            mu2r, rs2r = ln_finish(mean_b, sq_b, psE, "b")
            with tc.tile_pool(name="outp", bufs=4) as outp:
                for dt in range(NDT):
                    nc.vector.tensor_sub(z[dt][:], z[dt][:], mu2r[:])
                    nc.vector.tensor_mul(z[dt][:], z[dt][:], rs2r[:])
                    ot = outp.tile([P, T], bf, tag="ot", bufs=4,
                                   name=f"ot{dt}")
                    nc.scalar.activation(
                        ot[:], z[dt][:], AF.Identity,
                        bias=elnb_sb[:, dt:dt + 1],
                        scale=elng_sb[:, dt:dt + 1])
                    nc.sync.dma_start(out_d[dt], ot[:])

    nc.compile()
    return nc


def _get_program():
    global _PROGRAM
    if _PROGRAM is None:
        _PROGRAM = _build_program()
    return _PROGRAM


def _host_prep(inputs):
    """Shard + lay out inputs for each of the 8 cores."""
    x = np.asarray(inputs["x"], np.float32)
    Wq = np.asarray(inputs["Wq"], np.float32)
    bq = np.asarray(inputs["bq"], np.float32)
    Wk = np.asarray(inputs["Wk"], np.float32)
    bk = np.asarray(inputs["bk"], np.float32)
    Wv = np.asarray(inputs["Wv"], np.float32)
    bv = np.asarray(inputs["bv"], np.float32)
    scalar = np.float32(inputs["scalar"])
    ln_g = np.asarray(inputs["ln_g"], np.float32)
    ln_b = np.asarray(inputs["ln_b"], np.float32)
    eW1 = np.asarray(inputs["eW1"], np.float32)
    eb1 = np.asarray(inputs["eb1"], np.float32)
    eW2 = np.asarray(inputs["eW2"], np.float32)
    eb2 = np.asarray(inputs["eb2"], np.float32)
    e_scalar = np.asarray(inputs["e_scalar"], np.float32)
    eln_g = np.asarray(inputs["eln_g"], np.float32)
    eln_b = np.asarray(inputs["eln_b"], np.float32)

    # x pair-tiled: xp[b, p, f, j, t] = x[b, t, 256p + 128j + f]
    xT = x.transpose(0, 2, 1)                      # [B, D, S]
    xp = np.ascontiguousarray(
        xT.reshape(B, NPD, 2, P, S).transpose(0, 1, 3, 2, 4)).astype(F8NP)
    tri = (np.arange(P)[None, :] >= np.arange(P)[:, None]).astype(F8NP)

    def col(v):
        return np.ascontiguousarray(v.reshape(-1, 1), dtype=np.float32)

    def pk(v):  # [D]-like -> [P, n]
        n = v.size // P
        return np.ascontiguousarray(v.reshape(n, P).T, dtype=np.float32)

    def pair_w(w):  # [K, M] -> [K/256, P, 2, M] (pairs along contraction)
        M = w.shape[1]
        return np.ascontiguousarray(
            w.reshape(-1, 2, P, M).transpose(0, 2, 1, 3)).astype(F8NP)

    in_maps = []
    for c in range(NCORES):
        h0 = 2 * c
        b_out, e_out = c // NQB, c % NQB
        t0 = e_out * T
        wq_c = np.concatenate([Wq[h0], Wq[h0 + 1]], axis=1)  # [1024,128]
        wk_c = np.concatenate([Wk[h0], Wk[h0 + 1]], axis=1)
        wv_c = np.concatenate([Wv[h0], Wv[h0 + 1]], axis=1)
        bq_c = np.concatenate([bq[h0], bq[h0 + 1]])
        bk_c = np.concatenate([bk[h0], bk[h0 + 1]])
        bv_c = np.concatenate([bv[h0], bv[h0 + 1]])
        xc = np.ascontiguousarray(x[b_out, t0:t0 + T, :].T)  # [1024, 512]
        w1g = ln_g[:, None] * eW1[e_out]              # fold ln1 gamma
        b1f = eb1[e_out] + ln_b @ eW1[e_out]          # fold ln1 beta
        zb = ln_b + e_scalar[e_out] * eb2[e_out]      # ln1 beta + es*b2
        m = {
            "xp": xp,
            "wq": pair_w(wq_c),
            "wk": pair_w(wk_c),
            "wv": pair_w(wv_c),
            "bq": col(bq_c),
            "bk": col(bk_c),
            "bvg16": col(PRE * scalar * bv_c),
            "gate16": np.full((P, 1), PRE * scalar, np.float32),
            "tri": tri,
            "onesc_b": np.ones((P, 1), BF16NP),
            "onesr_f": np.ones((1, P), np.float32),
            "xcT": np.ascontiguousarray(xc.reshape(NDT, P, T), np.float32),
            "lng": pk(ln_g),
            "zbias": pk(zb),
            "w1": pair_w(w1g),
            "b1": pk(b1f),
            "w2": pair_w(eW2[e_out]),
            "es": np.full((P, 1), e_scalar[e_out], np.float32),
            "elng": pk(eln_g[e_out]),
            "elnb": pk(eln_b[e_out]),
        }
        in_maps.append(m)
    return in_maps


def _assemble(chunks):
    """chunks[c] = raw per-core 'out' [NDT, P, T] (feature-major bf16)."""
    out = np.empty((B, S, D), np.float32)
    for c in range(NCORES):
        b_out, e_out = c // NQB, c % NQB
        arr = np.asarray(chunks[c], np.float32).reshape(NDT, P, T)
        out[b_out, e_out * T:(e_out + 1) * T, :] = \
            arr.transpose(2, 0, 1).reshape(T, D)
    return out


_LAST_RESULT = {}


def kernel(**inputs) -> np.ndarray:
    import os
    from concourse.bass_utils import run_bass_kernel_spmd

    nc = _get_program()
    in_maps = _host_prep(inputs)
    trace = bool(int(os.environ.get("KBENCH_TRACE", "0")))
    res = run_bass_kernel_spmd(nc, in_maps, core_ids=list(range(NCORES)),
                               trace=trace)
    _LAST_RESULT["exec_time_ns"] = res.exec_time_ns
    _LAST_RESULT["res"] = res

    return _assemble([res.results[c]["out"] for c in range(NCORES)])


# revision 14
# speedup vs baseline: 1.5976x; 1.0446x over previous
"""Distributed Trainium2 kernel for AttentionLayer+Experts (fp8 rebuild).

Model: B=2, S=2048, D=1024, H=16 heads (DA=64), causal attention with
custom 1/(sqrt(64)*12) scale, residual gate, LayerNorm, then 4
sequence-chunk experts (FFN 1024->4096->1024, exact gelu), residual
with per-expert scalar, per-expert LayerNorm.

Sharding over 8 NeuronCores:
  - Attention head-parallel (core c owns heads 2c, 2c+1 for both
    batches); AllToAll converts head-sharding -> sequence-sharding so
    core c ends up with (batch c//4, seq chunk c%4) = one expert chunk.

Perf design:
  - All wide-contraction matmuls (QKV projections, AV, FFN1, FFN2) in
    fp8e4m3 with MatmulPerfMode.DoubleRow: two 128-row k-subtiles per
    instruction at 2x rate, operands in [128, 2, n] pair tiles.
    Scores stay bf16 (64-wide contraction cannot pair).
  - The attention sweep is ScalarE(exp)-bound, so batch 1's projections
    are emitted interleaved with batch 0's h=0 score/exp blocks: exp
    starts ~25us earlier and the PE stays fed from the in-order queue.
  - AllToAll split by head parity (first hides under the h=1 sweep),
    fp8 payload with 16x prescale (raw values sit at e4m3's subnormal
    edge).
  - LN1 gamma/beta folded into W1/b1 and the z-residual; LN activations
    and stats in bf16; mean+sumsq share one matmul via [P,2,T] tiles
    holding (x, x^2).
  - Output leaves feature-major bf16; host transposes.
"""

import numpy as np
import ml_dtypes

BF16NP = ml_dtypes.bfloat16
F8NP = ml_dtypes.float8_e4m3

B, S, D, H, DA, E = 2, 2048, 1024, 16, 64, 4
DFF = 4 * D
NCORES = 8
T = S // E        # 512 tokens per chunk / core
P = 128
SCALE = 1.0 / (np.sqrt(DA) * 12.0)
EPS = 1e-5
NDT = D // P      # 8 feature tiles
NPD = NDT // 2    # 4 feature pair-tiles
NQB = S // 512    # 4 query blocks per batch
NKT = S // P      # 16 key tiles per batch
NM1 = DFF // P    # 32 dff tiles
NM1P = NM1 // 2   # 16 dff pair-tiles
PRE = 16.0        # a2a fp8 prescale

_PROGRAM = None


def _build_program():
    from contextlib import ExitStack
    import concourse.bass as bass
    import concourse.mybir as mybir
    import concourse.tile as tile
    from concourse import bacc

    f32 = mybir.dt.float32
    bf = mybir.dt.bfloat16
    f8 = mybir.dt.float8e4
    AF = mybir.ActivationFunctionType
    ALU = mybir.AluOpType
    DR = mybir.MatmulPerfMode.DoubleRow

    nc = bacc.Bacc("TRN2", target_bir_lowering=False, debug=False,
                   num_devices=NCORES)

    def din(name, shape, dt):
        return nc.dram_tensor(name, shape, dt, kind="ExternalInput").ap()

    xp_d = din("xp", [B, NPD, P, 2, S], f8)      # x pair-tiled, both batches
    wq = din("wq", [P, NPD, 2, P], f8)           # SBUF layout on host
    wk = din("wk", [P, NPD, 2, P], f8)
    wv = din("wv", [P, NPD, 2, P], f8)
    bqv = din("bq", [P, 1], f32)
    bkv = din("bk", [P, 1], f32)
    bvg16 = din("bvg16", [P, 1], f32)            # PRE * gate * bv
    gate16 = din("gate16", [P, 1], f32)          # PRE * gate
    tri = din("tri", [P, P], f8)                 # tri[p,f] = f>=p
    onesc_b = din("onesc_b", [P, 1], bf)
    onesr_f = din("onesr_f", [1, P], f32)
    xcT = din("xcT", [NDT, P, T], f32)           # residual x^T for my chunk
    lng = din("lng", [P, NDT], f32)              # ln1 gamma (per dt col)
    zbias = din("zbias", [P, NDT], f32)          # ln1 beta + es*b2
    w1 = din("w1", [NPD, P, 2, DFF], f8)         # g-folded W1 pair tiles
    b1v = din("b1", [P, NM1], f32)               # b1 + ln1beta @ W1
    w2 = din("w2", [NM1P, P, 2, D], f8)
    esv = din("es", [P, 1], f32)                 # e_scalar replicated
    elng = din("elng", [P, NDT], f32)
    elnb = din("elnb", [P, NDT], f32)
    out_d = nc.dram_tensor("out", [NDT, P, T], bf, kind="ExternalOutput").ap()

    with tile.TileContext(nc) as tc, ExitStack() as ctx:
        cpool = ctx.enter_context(tc.tile_pool(name="const", bufs=1))
        wpool = ctx.enter_context(tc.tile_pool(name="wpool", bufs=1))
        xcp = ctx.enter_context(tc.tile_pool(name="xcp", bufs=NDT))
        dpool = ctx.enter_context(
            tc.tile_pool(name="dramp", bufs=1, space="DRAM"))
        qkp_ctx = ExitStack()
        qkp = qkp_ctx.enter_context(tc.tile_pool(name="qkp", bufs=4))
        vp_ctx = ExitStack()
        vp = vp_ctx.enter_context(tc.tile_pool(name="vp", bufs=NKT))
        xtp_ctx = ExitStack()
        xtp = xtp_ctx.enter_context(tc.tile_pool(name="xtp", bufs=2 * NPD))

        # ---- attention-phase inputs first (DMA priority) ----
        wq_sb = cpool.tile([P, NPD, 2, P], f8)
        nc.sync.dma_start(wq_sb[:], wq[:])
        wk_sb = cpool.tile([P, NPD, 2, P], f8)
        nc.sync.dma_start(wk_sb[:], wk[:])
        wv_sb = cpool.tile([P, NPD, 2, P], f8)
        nc.sync.dma_start(wv_sb[:], wv[:])
        bq_sb = cpool.tile([P, 1], f32)
        nc.sync.dma_start(bq_sb[:], bqv[:])
        bk_sb = cpool.tile([P, 1], f32)
        nc.sync.dma_start(bk_sb[:], bkv[:])
        bvg_sb = cpool.tile([P, 1], f32)
        nc.sync.dma_start(bvg_sb[:], bvg16[:])
        gate_sb = cpool.tile([P, 1], f32)
        nc.sync.dma_start(gate_sb[:], gate16[:])
        tri_sb = cpool.tile([P, P], f8)
        nc.sync.dma_start(tri_sb[:], tri[:])
        xt_all = {}
        for b in range(B):
            for pt in range(NPD):
                t = xtp.tile([P, 2, S], f8, tag="xt", bufs=2 * NPD,
                             name=f"xt{b}_{pt}")
                nc.sync.dma_start(t[:], xp_d[b, pt])
                xt_all[(b, pt)] = t

        # ---- later-phase constants + FFN weight prefetch ----
        onescb_sb = cpool.tile([P, 1], bf)
        nc.sync.dma_start(onescb_sb[:], onesc_b[:])
        onesrf_sb = cpool.tile([1, P], f32)
        nc.sync.dma_start(onesrf_sb[:], onesr_f[:])
        lng_sb = cpool.tile([P, NDT], f32)
        nc.sync.dma_start(lng_sb[:], lng[:])
        zbias_sb = cpool.tile([P, NDT], f32)
        nc.sync.dma_start(zbias_sb[:], zbias[:])
        b1_sb = cpool.tile([P, NM1], f32)
        nc.sync.dma_start(b1_sb[:], b1v[:])
        es_sb = cpool.tile([P, 1], f32)
        nc.sync.dma_start(es_sb[:], esv[:])
        elng_sb = cpool.tile([P, NDT], f32)
        nc.sync.dma_start(elng_sb[:], elng[:])
        elnb_sb = cpool.tile([P, NDT], f32)
        nc.sync.dma_start(elnb_sb[:], elnb[:])
        eps_sb = cpool.tile([1, 1], f32)
        nc.vector.memset(eps_sb[:], float(EPS))
        xc_sb = []
        for dt in range(NDT):
            t = xcp.tile([P, T], f32, tag="xc", bufs=NDT, name=f"xc{dt}")
            nc.sync.dma_start(t[:], xcT[dt])
            xc_sb.append(t)
        w1_sb = []
        for pt in range(NPD):
            t = wpool.tile([P, 2, DFF], f8, tag="w1", bufs=NPD,
                           name=f"w1_{pt}")
            nc.sync.dma_start(t[:], w1[pt])
            w1_sb.append(t)
        w2_sb = []
        for kp in range(NM1P):
            t = wpool.tile([P, 2, D], f8, tag="w2", bufs=NM1P,
                           name=f"w2_{kp}")
            nc.sync.dma_start(t[:], w2[kp])
            w2_sb.append(t)

        # a2a DRAM bounce buffers (split by head parity, fp8 payload)
        a_in = [dpool.tile([NCORES, 64, 512], f8, name=f"a_in{h}")
                for h in range(2)]
        a_out = [dpool.tile([NCORES, 64, 512], f8, name=f"a_out{h}")
                 for h in range(2)]

        # ======== proj + attention share one PSUM pool:
        # pj bufs=3 + sc bufs=3 + o bufs=2 -> exactly 8 banks ========
        qTs, kTs, vs = {}, {}, {}
        with tc.tile_pool(name="psA", bufs=1,
                          space=bass.MemorySpace.PSUM) as psA, \
             tc.tile_pool(name="ep", bufs=26) as epool, \
             tc.tile_pool(name="stgp", bufs=3) as stgp:

            for b in range(B):
                qTs[b] = qkp.tile([P, S], bf, tag="qT", bufs=2,
                                  name=f"qT{b}")
                kTs[b] = qkp.tile([P, S], bf, tag="kT", bufs=2,
                                  name=f"kT{b}")
                vs[b] = []
                for ktp in range(NKT // 2):
                    vt = vp.tile([P, 2, 2 * P], f8, tag="v", bufs=NKT,
                                 name=f"v{b}_{ktp}")
                    nc.vector.memset(vt[:], 1.0)
                    vs[b].append(vt)

            def proj_qk(b, qb):
                q0 = 512 * qb
                for (w_sb, b_sb, oT) in ((wq_sb, bq_sb, qTs[b]),
                                         (wk_sb, bk_sb, kTs[b])):
                    ps = psA.tile([P, 512], f32, tag="pj", bufs=3,
                                  name=f"pj{b}{qb}{w_sb is wk_sb}")
                    for pt in range(NPD):
                        nc.tensor.matmul(
                            ps[:], w_sb[:, pt],
                            xt_all[(b, pt)][:, :, q0:q0 + 512],
                            start=(pt == 0), stop=(pt == NPD - 1),
                            perf_mode=DR)
                    nc.vector.tensor_scalar_add(
                        oT[:, q0:q0 + 512], ps[:], b_sb[:])

            def proj_v(b, tt):
                t0 = P * tt
                pv = psA.tile([P, 512], f32, tag="pj", bufs=3,
                              name=f"pv{b}{tt}")
                for pt in range(NPD):
                    nc.tensor.matmul(
                        pv[:, 0:P],
                        xt_all[(b, pt)][:, :, t0:t0 + P], wv_sb[:, pt],
                        start=(pt == 0), stop=(pt == NPD - 1),
                        perf_mode=DR)
                vt = vs[b][tt // 2]
                nc.vector.tensor_copy(vt[:, tt % 2, 0:64], pv[:, 0:64])
                nc.vector.tensor_copy(vt[:, tt % 2, P:P + 64],
                                      pv[:, 64:128])

            def sc_exp(h, b, qb):
                """Score + exp for every key tile of one query block.
                Returns the e_pair tiles for a later av()."""
                hp = h * 64
                q0 = 512 * qb
                qT, kT = qTs[b], kTs[b]
                pairs = []
                for ktp in range(2 * qb + 2):
                    e_pair = epool.tile([P, 2, 512], f8, tag="exp",
                                        bufs=26, name=f"e{b}{qb}{h}{ktp}")
                    for j in range(2):
                        kt = 2 * ktp + j
                        k0 = P * kt
                        off = max(0, k0 - q0)
                        s_ps = psA.tile([P, 512], f32, tag="sc", bufs=3,
                                        name=f"s{b}{qb}{h}{kt}")
                        nc.tensor.matmul(
                            s_ps[:, off:512],
                            kT[hp:hp + 64, k0:k0 + P],
                            qT[hp:hp + 64, q0 + off:q0 + 512],
                            start=True, stop=True)
                        if off:
                            nc.vector.memset(e_pair[:, j, 0:off], 0.0)
                        nc.scalar.activation(
                            e_pair[:, j, off:512], s_ps[:, off:512],
                            AF.Exp, bias=0.0, scale=float(SCALE))
                        if k0 >= q0:  # diagonal block: causal mask
                            nc.vector.tensor_mul(
                                e_pair[:, j, off:off + P],
                                e_pair[:, j, off:off + P], tri_sb[:])
                    pairs.append(e_pair)
                return pairs

            def av_stage(h, b, qb, pairs):
                hp = h * 64
                npair = len(pairs)
                o_ps = psA.tile([P, 512], f32, tag="o", bufs=2,
                                name=f"o{b}{qb}{h}")
                for ktp, e_pair in enumerate(pairs):
                    nc.tensor.matmul(
                        o_ps[:],
                        vs[b][ktp][:, :, h * 2 * 64:h * 2 * 64 + P],
                        e_pair[:],
                        start=(ktp == 0), stop=(ktp == npair - 1),
                        perf_mode=DR)
                # rowsum -> SBUF (reciprocal seed needs IEEE fp32)
                rsum = epool.tile([64, 512], f32, tag="rsum",
                                  bufs=2, name=f"rw{b}{qb}{h}")
                nc.vector.tensor_copy(rsum[:], o_ps[64:128, :])
                recip = epool.tile([64, 512], f32, tag="recip",
                                   bufs=2, name=f"rc{b}{qb}{h}")
                nc.vector.reciprocal_approx_fast(recip[:], rsum[:])
                stgb = stgp.tile([64, 512], bf, tag="stgb", bufs=3,
                                 name=f"sb{b}{qb}{h}")
                # stage = (o * 16gate) * (1/rowsum) + 16*gate*bv
                nc.vector.scalar_tensor_tensor(
                    stgb[:], o_ps[0:64, :],
                    gate_sb[0:64, :], recip[:], ALU.mult, ALU.mult)
                stg = stgp.tile([64, 512], f8, tag="stg", bufs=3,
                                name=f"stg{b}{qb}{h}")
                nc.vector.tensor_scalar_add(
                    stg[:], stgb[:], bvg_sb[hp:hp + 64, :])
                nc.sync.dma_start(a_in[h][b * NQB + qb], stg[:])

            def sweep(h, b, qb):
                av_stage(h, b, qb, sc_exp(h, b, qb))

            # ---- emission schedule ----
            for qb in range(NQB):
                proj_qk(0, qb)
            for tt in range(NKT):
                proj_v(0, tt)
            # interleave: (h0,b0) score/exp blocks feed ScalarE while the
            # PE chews batch-1 projections behind them in the queue
            pairs00 = [None] * NQB
            pairs00[0] = sc_exp(0, 0, 0)
            proj_qk(1, 0)
            pairs00[1] = sc_exp(0, 0, 1)
            proj_qk(1, 1)
            pairs00[2] = sc_exp(0, 0, 2)
            proj_qk(1, 2)
            for tt in range(0, 8):
                proj_v(1, tt)
            pairs00[3] = sc_exp(0, 0, 3)
            proj_qk(1, 3)
            for tt in range(8, NKT):
                proj_v(1, tt)
            for qb in range(NQB):
                av_stage(0, 0, qb, pairs00[qb])
            for qb in range(NQB):
                sweep(0, 1, qb)
            nc.gpsimd.collective_compute(
                "AllToAll", mybir.AluOpType.bypass,
                replica_groups=[list(range(NCORES))],
                ins=[a_in[0][:].opt()], outs=[a_out[0][:].opt()])
            for b in range(B):
                for qb in range(NQB):
                    sweep(1, b, qb)
            nc.gpsimd.collective_compute(
                "AllToAll", mybir.AluOpType.bypass,
                replica_groups=[list(range(NCORES))],
                ins=[a_in[1][:].opt()], outs=[a_out[1][:].opt()])
        xtp_ctx.close()
        vp_ctx.close()
        qkp_ctx.close()

        # =========== phase 3: residual + LN1 (gamma/beta folded) ==========
        # st[dt] is [P, 2, T] bf16: slot 0 = x1 (-> u after norm),
        # slot 1 = x1^2; one matmul accumulates mean and sumsq together.
        lnp = ctx.enter_context(tc.tile_pool(name="lnp", bufs=1))
        aop = ctx.enter_context(tc.tile_pool(name="aop", bufs=4))
        smp2 = ctx.enter_context(tc.tile_pool(name="smp2", bufs=1))
        st = [lnp.tile([P, 2, T], bf, tag="st", bufs=NDT, name=f"st{dt}")
              for dt in range(NDT)]
        x1p = [lnp.tile([P, 2, T], f8, tag="x1p", bufs=NPD, name=f"x1p{pt}")
               for pt in range(NPD)]

        def ln_finish(mean_ps, sq_ps, psum_pool, nm):
            """Replicated mu/rsig PSUM tiles from accumulated stats."""
            mu = smp2.tile([1, 512], f32, tag="sm2", bufs=8, name=f"mu{nm}")
            nc.vector.tensor_scalar_mul(mu[:], mean_ps[:], 1.0 / D)
            ex2 = smp2.tile([1, 512], f32, tag="sm2", bufs=8, name=f"e2{nm}")
            nc.vector.tensor_scalar_mul(ex2[:], sq_ps[:], 1.0 / D)
            mu2 = smp2.tile([1, 512], f32, tag="sm2", bufs=8, name=f"m2{nm}")
            nc.vector.tensor_mul(mu2[:], mu[:], mu[:])
            var = smp2.tile([1, 512], f32, tag="sm2", bufs=8, name=f"vr{nm}")
            nc.vector.tensor_sub(var[:], ex2[:], mu2[:])
            sig = smp2.tile([1, 512], f32, tag="sm2", bufs=8, name=f"sg{nm}")
            nc.scalar.activation(sig[:], var[:], AF.Sqrt, bias=eps_sb[:])
            rsig = smp2.tile([1, 512], f32, tag="sm2", bufs=8,
                             name=f"rs{nm}")
            nc.vector.reciprocal_approx_fast(rsig[:], sig[:])
            mu_rep = psum_pool.tile([P, 512], f32, tag="rep2", bufs=2,
                                    name=f"mr{nm}")
            nc.tensor.matmul(mu_rep[:], onesrf_sb[:], mu[:],
                             start=True, stop=True)
            rs_rep = psum_pool.tile([P, 512], f32, tag="rep2", bufs=2,
                                    name=f"rr{nm}")
            nc.tensor.matmul(rs_rep[:], onesrf_sb[:], rsig[:],
                             start=True, stop=True)
            return mu_rep, rs_rep

        with tc.tile_pool(name="psB", bufs=1,
                          space=bass.MemorySpace.PSUM) as psB:
            mean_a = psB.tile([1, 512], f32, tag="red", bufs=2, name="mna")
            sq_a = psB.tile([1, 512], f32, tag="red", bufs=2, name="sqa")
            for dt in range(NDT):
                ao = aop.tile([P, 512], f8, tag="ao", name=f"ao{dt}")
                nc.sync.dma_start(ao[0:64, :], a_out[0][dt])
                nc.sync.dma_start(ao[64:128, :], a_out[1][dt])
                # x1 = xc + stage/16
                nc.vector.scalar_tensor_tensor(
                    st[dt][:, 0, :], ao[:], 1.0 / PRE, xc_sb[dt][:],
                    ALU.mult, ALU.add)
                nc.vector.tensor_mul(st[dt][:, 1, :], st[dt][:, 0, :],
                                     st[dt][:, 0, :])
                nc.tensor.matmul(mean_a[:], onescb_sb[:], st[dt][:, 0, :],
                                 start=(dt == 0), stop=(dt == NDT - 1))
                nc.tensor.matmul(sq_a[:], onescb_sb[:], st[dt][:, 1, :],
                                 start=(dt == 0), stop=(dt == NDT - 1))
            mu_rep, rs_rep = ln_finish(mean_a, sq_a, psB, "a")
            for dt in range(NDT):
                u = st[dt][:, 0, :]
                nc.vector.tensor_sub(u, u, mu_rep[:])
                nc.vector.tensor_mul(u, u, rs_rep[:])
                nc.vector.tensor_copy(x1p[dt // 2][:, dt % 2, :], u)

            # =========== phase 4: expert FFN1 (fp8 DR) ==========
            hp_pool = ctx.enter_context(tc.tile_pool(name="hT", bufs=NM1P))
            hT = [hp_pool.tile([P, 2, T], f8, tag="hT", name=f"hT{kp}")
                  for kp in range(NM1P)]
            with tc.tile_pool(name="psC", bufs=1,
                              space=bass.MemorySpace.PSUM) as psC:
                for mg in range(11):
                    ms = range(3 * mg, min(3 * mg + 3, NM1))
                    fps = {m: psC.tile([P, T], f32, tag="f1", bufs=3,
                                       name=f"f1_{m}") for m in ms}
                    for pt in range(NPD):
                        for m in ms:
                            nc.tensor.matmul(
                                fps[m][:],
                                w1_sb[pt][:, :, m * P:(m + 1) * P],
                                x1p[pt][:],
                                start=(pt == 0), stop=(pt == NPD - 1),
                                perf_mode=DR)
                    for m in ms:
                        nc.scalar.activation(
                            hT[m // 2][:, m % 2, :], fps[m][:], AF.Gelu,
                            bias=b1_sb[:, m:m + 1], scale=1.0)

        # =========== phase 5: FFN2 (fp8 DR) + LN2 ==========
        # zst[dt]: [P, 2, T] bf16 with (z, z^2), like LN1
        zst = [lnp.tile([P, 2, T], bf, tag="zst", bufs=NDT, name=f"zs{dt}")
               for dt in range(NDT)]
        with tc.tile_pool(name="psE", bufs=1,
                          space=bass.MemorySpace.PSUM) as psE:
            mean_b = psE.tile([1, 512], f32, tag="red", bufs=2, name="mnb")
            sq_b = psE.tile([1, 512], f32, tag="red", bufs=2, name="sqb")
            with tc.tile_pool(name="psD", bufs=1,
                              space=bass.MemorySpace.PSUM) as psD:
                for dg in range(3):
                    dts = range(3 * dg, min(3 * dg + 3, NDT))
                    yps = {dt: psD.tile([P, T], f32, tag="f2", bufs=3,
                                        name=f"y{dt}") for dt in dts}
                    for kp in range(NM1P):
                        for dt in dts:
                            nc.tensor.matmul(
                                yps[dt][:],
                                w2_sb[kp][:, :, dt * P:(dt + 1) * P],
                                hT[kp][:],
                                start=(kp == 0), stop=(kp == NM1P - 1),
                                perf_mode=DR)
                    for dt in dts:
                        # z = es*y + (ln1b + es*b2) + ln1g*u
                        tz = smp2.tile([P, T], bf, tag="tz", bufs=3,
                                       name=f"tz{dt}")
                        nc.scalar.activation(
                            tz[:], yps[dt][:], AF.Identity,
                            bias=zbias_sb[:, dt:dt + 1],
                            scale=es_sb[:])
                        zt = zst[dt][:, 0, :]
                        nc.vector.scalar_tensor_tensor(
                            zt, st[dt][:, 0, :], lng_sb[:, dt:dt + 1],
                            tz[:], ALU.mult, ALU.add)
                        nc.vector.tensor_mul(zst[dt][:, 1, :], zt, zt)
                        nc.tensor.matmul(mean_b[:], onescb_sb[:],
                                         zst[dt][:, 0, :],
                                         start=(dt == 0),
                                         stop=(dt == NDT - 1))
                        nc.tensor.matmul(sq_b[:], onescb_sb[:],
                                         zst[dt][:, 1, :],
                                         start=(dt == 0),
                                         stop=(dt == NDT - 1))

            # =========== phase 6: LN2 + output (feature-major) ==========
            mu2r, rs2r = ln_finish(mean_b, sq_b, psE, "b")
            with tc.tile_pool(name="outp", bufs=4) as outp:
                for dt in range(NDT):
                    zt = zst[dt][:, 0, :]
                    nc.vector.tensor_sub(zt, zt, mu2r[:])
                    nc.vector.tensor_mul(zt, zt, rs2r[:])
                    ot = outp.tile([P, T], bf, tag="ot", bufs=4,
                                   name=f"ot{dt}")
                    nc.scalar.activation(
                        ot[:], zt, AF.Identity,
                        bias=elnb_sb[:, dt:dt + 1],
                        scale=elng_sb[:, dt:dt + 1])
                    nc.sync.dma_start(out_d[dt], ot[:])

    nc.compile()
    return nc


def _get_program():
    global _PROGRAM
    if _PROGRAM is None:
        _PROGRAM = _build_program()
    return _PROGRAM


def _host_prep(inputs):
    """Shard + lay out inputs for each of the 8 cores."""
    x = np.asarray(inputs["x"], np.float32)
    Wq = np.asarray(inputs["Wq"], np.float32)
    bq = np.asarray(inputs["bq"], np.float32)
    Wk = np.asarray(inputs["Wk"], np.float32)
    bk = np.asarray(inputs["bk"], np.float32)
    Wv = np.asarray(inputs["Wv"], np.float32)
    bv = np.asarray(inputs["bv"], np.float32)
    scalar = np.float32(inputs["scalar"])
    ln_g = np.asarray(inputs["ln_g"], np.float32)
    ln_b = np.asarray(inputs["ln_b"], np.float32)
    eW1 = np.asarray(inputs["eW1"], np.float32)
    eb1 = np.asarray(inputs["eb1"], np.float32)
    eW2 = np.asarray(inputs["eW2"], np.float32)
    eb2 = np.asarray(inputs["eb2"], np.float32)
    e_scalar = np.asarray(inputs["e_scalar"], np.float32)
    eln_g = np.asarray(inputs["eln_g"], np.float32)
    eln_b = np.asarray(inputs["eln_b"], np.float32)

    # x pair-tiled: xp[b, p, f, j, t] = x[b, t, 256p + 128j + f]
    xT = x.transpose(0, 2, 1)                      # [B, D, S]
    xp = np.ascontiguousarray(
        xT.reshape(B, NPD, 2, P, S).transpose(0, 1, 3, 2, 4)).astype(F8NP)
    tri = (np.arange(P)[None, :] >= np.arange(P)[:, None]).astype(F8NP)

    def col(v):
        return np.ascontiguousarray(v.reshape(-1, 1), dtype=np.float32)

    def pk(v):  # [D]-like -> [P, n]
        n = v.size // P
        return np.ascontiguousarray(v.reshape(n, P).T, dtype=np.float32)

    def pair_w(w):  # [K, M] -> [K/256, P, 2, M] (pairs along contraction)
        M = w.shape[1]
        return np.ascontiguousarray(
            w.reshape(-1, 2, P, M).transpose(0, 2, 1, 3)).astype(F8NP)

    def pair_w_sb(w):  # [D, 128] -> [P, NPD, 2, 128] (SBUF layout)
        return np.ascontiguousarray(
            w.reshape(NPD, 2, P, P).transpose(2, 0, 1, 3)).astype(F8NP)

    in_maps = []
    for c in range(NCORES):
        h0 = 2 * c
        b_out, e_out = c // NQB, c % NQB
        t0 = e_out * T
        wq_c = np.concatenate([Wq[h0], Wq[h0 + 1]], axis=1)  # [1024,128]
        wk_c = np.concatenate([Wk[h0], Wk[h0 + 1]], axis=1)
        wv_c = np.concatenate([Wv[h0], Wv[h0 + 1]], axis=1)
        bq_c = np.concatenate([bq[h0], bq[h0 + 1]])
        bk_c = np.concatenate([bk[h0], bk[h0 + 1]])
        bv_c = np.concatenate([bv[h0], bv[h0 + 1]])
        xc = np.ascontiguousarray(x[b_out, t0:t0 + T, :].T)  # [1024, 512]
        w1g = ln_g[:, None] * eW1[e_out]              # fold ln1 gamma
        b1f = eb1[e_out] + ln_b @ eW1[e_out]          # fold ln1 beta
        zb = ln_b + e_scalar[e_out] * eb2[e_out]      # ln1 beta + es*b2
        m = {
            "xp": xp,
            "wq": pair_w_sb(wq_c),
            "wk": pair_w_sb(wk_c),
            "wv": pair_w_sb(wv_c),
            "bq": col(bq_c),
            "bk": col(bk_c),
            "bvg16": col(PRE * scalar * bv_c),
            "gate16": np.full((P, 1), PRE * scalar, np.float32),
            "tri": tri,
            "onesc_b": np.ones((P, 1), BF16NP),
            "onesr_f": np.ones((1, P), np.float32),
            "xcT": np.ascontiguousarray(xc.reshape(NDT, P, T), np.float32),
            "lng": pk(ln_g),
            "zbias": pk(zb),
            "w1": pair_w(w1g),
            "b1": pk(b1f),
            "w2": pair_w(eW2[e_out]),
            "es": np.full((P, 1), e_scalar[e_out], np.float32),
            "elng": pk(eln_g[e_out]),
            "elnb": pk(eln_b[e_out]),
        }
        in_maps.append(m)
    return in_maps


def _assemble(chunks):
    """chunks[c] = raw per-core 'out' [NDT, P, T] (feature-major bf16)."""
    out = np.empty((B, S, D), np.float32)
    for c in range(NCORES):
        b_out, e_out = c // NQB, c % NQB
        arr = np.asarray(chunks[c], np.float32).reshape(NDT, P, T)
        out[b_out, e_out * T:(e_out + 1) * T, :] = \
            arr.transpose(2, 0, 1).reshape(T, D)
    return out


_LAST_RESULT = {}


def kernel(**inputs) -> np.ndarray:
    import os
    from concourse.bass_utils import run_bass_kernel_spmd

    nc = _get_program()
    in_maps = _host_prep(inputs)
    trace = bool(int(os.environ.get("KBENCH_TRACE", "0")))
    res = run_bass_kernel_spmd(nc, in_maps, core_ids=list(range(NCORES)),
                               trace=trace)
    _LAST_RESULT["exec_time_ns"] = res.exec_time_ns
    _LAST_RESULT["res"] = res

    return _assemble([res.results[c]["out"] for c in range(NCORES)])
